# revision 38
# baseline (speedup 1.0000x reference)
"""BilinearAttention Trainium2 kernel — 8-core data-parallel (batch sharded).

Math per batch element b (reference semantics):
  d   = drug @ Wd + bd                     (N=128, HID=512)
  dWb = drug @ (Wd@Wb folded) + bdWb       (N, HID)     [host-folded weights]
  t   = target @ Wt + bt                   (L=1024, HID)
  per head h (HD=64):
    attn = dWb_h @ t_h^T                   (N, L)
    E    = exp(attn)  with masked rows/cols exactly 0 in the *inputs*
    d-side: p_d = E / rowsum(E);  w_d[l] = sum_n p_d * dm[n]/dlen
            ctx_d[h] = sum_l w_d[l] * t_h[l]
    t-side: p_t = E / colsum(E);  w_t[n] = sum_l p_t * tm[l]/tlen
            ctx_t[h] = sum_n w_t[n] * d_h[n]
  out[b] = [ctx_d(512) | ctx_t(512)]

Mask scheme: host zeroes invalid rows of drug/target; projection biases are
applied via rank-1 (bias x mask01) matmuls so projected features are exactly
0 at invalid positions.  exp(0)=1 at invalid attn entries; softmax
denominators are fixed by subtracting the host-known invalid counts.

Schedule notes:
 - prologue: drug projection for ALL 4 batch elements in one batched GEMM
   (starts ~2us in, warms the PE while tgtT(0) streams), then tproj(0)
   paced by per-k-chunk DMAs.
 - steady state: tproj(b+1) interleaved into the (ACT-limited) E phase of
   batch b so the tensor engine never drains.
 - t-side: attn^T recomputed per 128-row l-chunk with a head-pair repacked
   rhs (free=256 matmuls); the per-(l,head) softmax scale g = tmw/colsum is
   folded into the w_t reduction as the matmul *stationary* operand (gdual:
   [g_h | g_h'] column-doubled), so no broadcast multiply over Et is needed.
   The resulting PSUM has garbage in the cross quadrants, which are never
   read.
"""

import numpy as np
import ml_dtypes

import concourse.bass as bass
import concourse.bacc as bacc
import concourse.mybir as mybir
from concourse.bass_utils import run_bass_kernel_spmd
from concourse import tile
from concourse.masks import make_identity
from concourse.tile_rust import add_dep_helper

NCORES = 8
B = 32
BC = B // NCORES          # 4 batch elements per core
N, L = 128, 1024
KD, KT = 256, 1280        # drug dim, target dim
HID, H, HD = 512, 8, 64
NKC_T = KT // 128         # 10 k-chunks for target proj
NKC_D = KD // 128         # 2 k-chunks for drug proj
NC4 = HID // 128          # 4 hid chunks (2 heads each)
FP32 = mybir.dt.float32
BF16 = mybir.dt.bfloat16
AF = mybir.ActivationFunctionType
ALU = mybir.AluOpType
BF16NP = ml_dtypes.bfloat16


def _body(tc, io):
    nc = tc.nc
    import contextlib
    es = contextlib.ExitStack()

    const = es.enter_context(tc.tile_pool(name="const", bufs=1))

    # ---- critical-path inputs first: the serial per-dma_start issue cost
    # (~0.65us each on the Sync queue) dominates the prologue, so the drug
    # projection's inputs go first and the bulk target loads are issued from
    # the GpSimd SWDGE queue in parallel.
    # drug + wd2 arrive host-packed in the exact SBUF layout: one contiguous
    # DMA each keeps the serial dma_start issue path short.
    drugT_all = const.tile([128, NKC_D * BC * N], BF16, tag="drugT")
    nc.sync.dma_start(out=drugT_all[:], in_=io["drug_pack"][:])
    wd2_all = const.tile([128, NKC_D * 2 * HID], BF16, tag="wd2all")
    crit_dmas = [nc.sync.dma_start(out=wd2_all[:], in_=io["wd2_pack"][:])]

    # rpack bf16 [1, 512 + 1024 + BC*1024 + BC*128]: bt | b2 | tmask | dmask
    rpack = const.tile([1, HID + 2 * HID + BC * L + BC * N], BF16, tag="rpack")
    nc.sync.dma_start(out=rpack[:], in_=io["rpack"][:])
    bt_row = rpack[:, 0:HID]
    b2_row = rpack[:, HID:HID + 2 * HID]
    tmask_row_t = [rpack[:, 3 * HID + b * L:3 * HID + (b + 1) * L]
                   for b in range(BC)]
    dmask_all = rpack[:, 3 * HID + BC * L:3 * HID + BC * L + BC * N]

    # cpack fp32 [128, 3*BC + BC*64]: dmwT | sdcorrT | stcorrT | tmw64 per b
    cpack = const.tile([128, 3 * BC + BC * 64], FP32, tag="cpack")
    nc.sync.dma_start(out=cpack[:], in_=io["cpack"][:])
    dmwT = cpack[:, 0:BC]
    sdcorrT = cpack[:, BC:2 * BC]
    stcorrT = cpack[:, 2 * BC:3 * BC]
    tmw64_t = [cpack[:, 3 * BC + b * 64:3 * BC + (b + 1) * 64]
               for b in range(BC)]

    # identity first: its iota must not queue behind gated DMA issues on
    # the gpsimd queue
    ident_f = const.tile([128, 128], FP32, tag="idf")
    make_identity(nc, ident_f[:])
    ctxT_all = const.tile([128, 128], FP32, tag="ctxall")

    # target-proj weights: per-k-chunk DMAs (SWDGE queue) so tproj(0) can
    # stream; the first chunks are ungated, the rest wait on the critical
    # drug-side loads and are issued last (a gated issue head-of-line
    # blocks the whole gpsimd queue).
    wt_all = const.tile([128, NKC_T * HID], BF16, tag="wtall")
    for kc in range(4):
        nc.gpsimd.dma_start(
            out=wt_all[:, kc * HID:(kc + 1) * HID],
            in_=io["wt"][kc * 128:(kc + 1) * 128, :])

    # d projections for all batches: d2T_all[:, ch*512 + b*128 + n]
    # ch 0..3 = d (value side), ch 4..7 = dW (bilinear-mapped, attn side)
    d2T_all = const.tile([128, 8 * BC * N], BF16, tag="d2Tall")
    # head-pair repacked dW with zero padding, per batch:
    # d2pair2[:, b*1024 + c*256 + s*128 + n]; valid rows s*64..s*64+63
    d2p2 = const.tile([128, BC * 2 * HID], BF16, tag="d2p2")

    # ---- pools ----
    tgtT_pool = es.enter_context(tc.tile_pool(name="tgtT", bufs=2))
    tT_pool = es.enter_context(tc.tile_pool(name="tT", bufs=2))
    e_pool = es.enter_context(tc.tile_pool(name="E", bufs=10))
    et_pool = es.enter_context(tc.tile_pool(name="Et", bufs=9))
    small = es.enter_context(tc.tile_pool(name="small", bufs=4))
    gd_pool = es.enter_context(tc.tile_pool(name="gd", bufs=9))
    junk = es.enter_context(tc.tile_pool(name="junk", bufs=3))
    ps_main = es.enter_context(tc.tile_pool(name="psm", bufs=2, space="PSUM"))
    ps_tp = es.enter_context(tc.tile_pool(name="pst", bufs=2, space="PSUM"))
    ps_acc = es.enter_context(tc.tile_pool(name="psa", bufs=1, space="PSUM"))

    # ---------- helpers ----------
    def load_tgtT(bb, chunked=False):
        t = tgtT_pool.tile([128, NKC_T * L], BF16, tag="tgtT",
                           name=f"tgtT_{bb}")
        if chunked:
            for kc in range(NKC_T):
                i = nc.gpsimd.dma_start(
                    out=t[:, kc * L:(kc + 1) * L],
                    in_=io["tgtT"][bb, kc * 128:(kc + 1) * 128, :])
                if kc >= 4:
                    for cd in crit_dmas:
                        add_dep_helper(
                            i.ins, cd.ins, sync=True,
                            reason="bulk tgt DMA after critical drug DMAs")
        else:
            for half in range(2):
                i = nc.sync.dma_start(
                    out=t[:].rearrange("p (kc l) -> p kc l", l=L)[
                        :, half * 5:(half + 1) * 5, :],
                    in_=io["tgtT"][bb, half * 5 * 128:(half + 1) * 5 * 128, :]
                    .rearrange("(kc p) l -> p kc l", p=128),
                )
                if bb == 1:
                    # don't let the b=1 prefetch starve the prologue DMAs
                    for cd in crit_dmas:
                        add_dep_helper(i.ins, cd.ins, sync=True,
                                       reason="b1 prefetch after critical")
        return t

    tproj_state = {}

    def tproj_group(bb, tgtT_t, slot):
        # slot 0..7 -> (c, lh); dedicated 1-bank psum per slot so target
        # projection never blocks the E/Et psum rotation
        c, lh = slot // 2, slot % 2
        ps = ps_tp.tile([128, 512], FP32, tag="pst", name=f"ps_tp_{bb}_{slot}")
        for kc in range(NKC_T):
            nc.tensor.matmul(
                ps[:],
                lhsT=wt_all[:, kc * HID + c * 128:kc * HID + (c + 1) * 128],
                rhs=tgtT_t[:, kc * L + lh * 512:kc * L + (lh + 1) * 512],
                start=(kc == 0), stop=False,
            )
        nc.tensor.matmul(
            ps[:],
            lhsT=bt_row[:, c * 128:(c + 1) * 128],
            rhs=tmask_row_t[bb][:, lh * 512:(lh + 1) * 512],
            start=False, stop=True,
        )
        if lh == 0:
            tproj_state["t"] = tT_pool.tile(
                [128, L], BF16, tag=f"tT{c}", name=f"tT_{bb}_{c}")
        t = tproj_state["t"]
        nc.scalar.copy(t[:, lh * 512:(lh + 1) * 512], ps[:])
        return t if lh == 1 else None

    # ---------- prologue ----------
    # drug projection, all 4 batches in one batched GEMM (free dim = b*n=512)
    for cp in range(4):            # ch pairs (0,1), (2,3), (4,5), (6,7)
        ps_d = ps_main.tile([128, 2 * 512], FP32, tag="psm",
                            name=f"ps_dp_{cp}")
        for half in range(2):
            ch = 2 * cp + half
            for kc in range(NKC_D):
                nc.tensor.matmul(
                    ps_d[:, half * 512:(half + 1) * 512],
                    lhsT=wd2_all[:, kc * 2 * HID + ch * 128:
                                 kc * 2 * HID + (ch + 1) * 128],
                    rhs=drugT_all[:, kc * 512:(kc + 1) * 512],
                    start=(kc == 0), stop=False,
                )
            nc.tensor.matmul(
                ps_d[:, half * 512:(half + 1) * 512],
                lhsT=b2_row[:, ch * 128:(ch + 1) * 128],
                rhs=dmask_all[:],
                start=False, stop=True,
            )
        nc.scalar.copy(d2T_all[:, cp * 1024:(cp + 1) * 1024], ps_d[:])

    # head-pair repacked dW (zero padded halves)
    nc.vector.memset(d2p2[:].bitcast(FP32), 0.0)
    for s in range(2):
        for bb in range(BC):
            nc.vector.tensor_copy(
                d2p2[s * 64:(s + 1) * 64, bb * 1024:(bb + 1) * 1024]
                .rearrange("q (c z) -> q c z", z=256)
                [:, :, s * 128:(s + 1) * 128],
                d2T_all[s * 64:(s + 1) * 64, 4 * BC * N:8 * BC * N]
                .rearrange("q (c z) -> q c z", z=512)
                [:, :, bb * 128:(bb + 1) * 128],
            )

    # PE warmup: dummy matmuls cover the DMA wait so HAM is at full clock
    # when real work lands (a cold PE runs at 1.2 instead of 2.4 GHz)
    ps_w = ps_tp.tile([128, 512], FP32, tag="pst", name="ps_warm")
    for _ in range(56):
        nc.tensor.matmul(ps_w[:, 0:128], lhsT=ident_f[:].bitcast(BF16)[:, 0:128],
                         rhs=ident_f[:].bitcast(BF16)[:, 0:128],
                         start=True, stop=True)

    # tproj(0) inputs: ungated tgt chunks first, then the gated remainder
    # of both wt and tgt
    tgtT_cur = tgtT_pool.tile([128, NKC_T * L], BF16, tag="tgtT",
                              name="tgtT_0")
    for kc in range(4):
        nc.gpsimd.dma_start(
            out=tgtT_cur[:, kc * L:(kc + 1) * L],
            in_=io["tgtT"][0, kc * 128:(kc + 1) * 128, :])
    for kc in range(4, NKC_T):
        i = nc.gpsimd.dma_start(
            out=wt_all[:, kc * HID:(kc + 1) * HID],
            in_=io["wt"][kc * 128:(kc + 1) * 128, :])
        for cd in crit_dmas:
            add_dep_helper(i.ins, cd.ins, sync=True,
                           reason="bulk wt DMA after critical drug DMAs")
        i = nc.gpsimd.dma_start(
            out=tgtT_cur[:, kc * L:(kc + 1) * L],
            in_=io["tgtT"][0, kc * 128:(kc + 1) * 128, :])
        for cd in crit_dmas:
            add_dep_helper(i.ins, cd.ins, sync=True,
                           reason="bulk tgt DMA after critical drug DMAs")

    # tproj(0): kc-outer passes over slot pairs so the PE consumes target
    # chunks as their DMAs land instead of waiting for all of them
    tT_cur = []
    for c in range(4):
        pspair = [ps_tp.tile([128, 512], FP32, tag="pst",
                             name=f"ps_tp0_{c}_{lh}") for lh in range(2)]
        for kc in range(NKC_T):
            for lh in range(2):
                nc.tensor.matmul(
                    pspair[lh][:],
                    lhsT=wt_all[:, kc * HID + c * 128:kc * HID + (c + 1) * 128],
                    rhs=tgtT_cur[:, kc * L + lh * 512:kc * L + (lh + 1) * 512],
                    start=(kc == 0), stop=False,
                )
        t = tT_pool.tile([128, L], BF16, tag=f"tT{c}", name=f"tT_0_{c}")
        for lh in range(2):
            nc.tensor.matmul(
                pspair[lh][:],
                lhsT=bt_row[:, c * 128:(c + 1) * 128],
                rhs=tmask_row_t[0][:, lh * 512:(lh + 1) * 512],
                start=False, stop=True,
            )
            nc.scalar.copy(t[:, lh * 512:(lh + 1) * 512], pspair[lh][:])
        tT_cur.append(t)

    # ---------- per-batch steady state ----------
    for b in range(BC):
        nxt = b + 1 if b + 1 < BC else None
        if nxt is not None:
            tgtT_nxt = load_tgtT(nxt)
        tT = tT_cur

        E = [e_pool.tile([128, L], BF16, tag="E", name=f"E_{b}_{i}")
             for i in range(H)]
        S_d8 = small.tile([128, 8], FP32, tag="Sd8")
        ctxv = small.tile([128, 8], FP32, tag="ctx")
        S_t = small.tile([128, 64], FP32, tag="St")
        ps_wt = ps_acc.tile([128, 1024], FP32, tag="psa", name=f"ps_wt_{b}")
        Et_tiles = []
        mm_bank_start = {}
        tT_nxt = []
        u_rep = small.tile([128, 8 * 64], BF16, tag="urep")

        def e_head(h):
            c, ph = h // 2, (h % 2) * 64
            ps = ps_main.tile([128, 2 * 512], FP32, tag="psm",
                              name=f"ps_E_{b}_{h}")
            for lh in range(2):
                nc.tensor.matmul(
                    ps[:, lh * 512:(lh + 1) * 512],
                    lhsT=d2T_all[ph:ph + 64,
                                 (4 + c) * 512 + b * 128:(4 + c) * 512 + (b + 1) * 128],
                    rhs=tT[c][ph:ph + 64, lh * 512:(lh + 1) * 512],
                    start=True, stop=True,
                )
            nc.scalar.activation(
                E[h][:], ps[:], AF.Exp,
                accum_out=S_d8[:, h:h + 1],
            )

        def u_chain():
            nc.vector.tensor_scalar(
                out=S_d8[:], in0=S_d8[:], scalar1=sdcorrT[:, b:b + 1],
                scalar2=None, op0=ALU.add,
            )
            recipSd = small.tile([128, 8], FP32, tag="rSd")
            nc.vector.reciprocal(recipSd[:], S_d8[:])
            u_f = small.tile([128, 8], FP32, tag="uf")
            nc.vector.tensor_scalar(
                out=u_f[:], in0=recipSd[:], scalar1=dmwT[:, b:b + 1],
                scalar2=None, op0=ALU.mult,
            )
            nc.vector.tensor_copy(
                u_rep[:].rearrange("p (h z) -> p h z", z=64),
                u_f[:, :, None].to_broadcast((128, 8, 64)),
            )

        def wd_group(c):
            ps = ps_main.tile([128, 2 * 512], FP32, tag="psm",
                              name=f"ps_wd_{b}_{c}")
            for lh in range(2):
                for hp in range(2):
                    h = 2 * c + hp
                    nc.tensor.matmul(
                        ps[hp * 64:(hp + 1) * 64, lh * 512:(lh + 1) * 512],
                        lhsT=u_rep[:, h * 64:(h + 1) * 64],
                        rhs=E[h][:, lh * 512:(lh + 1) * 512],
                        start=True, stop=True,
                    )
            scratch = junk.tile([128, 1024], BF16, tag="junk")
            nc.vector.scalar_tensor_tensor(
                out=scratch[:], in0=ps[:], scalar=1.0,
                in1=tT[c][:],
                op0=ALU.mult, op1=ALU.mult,
                accum_out=ctxv[:, c:c + 1],
            )

        def et_lc(lc, tail=False):
            Et = et_pool.tile([128, 1024], BF16, tag="Et", name=f"Et_{b}_{lc}")
            if tail:
                # no tproj filler in the tail: use its psum pool so this exp
                # chain pipelines independently of the E-head exp chain
                for hf in range(2):
                    psh = ps_tp.tile([128, 512], FP32, tag="pst",
                                     name=f"ps_Et_{b}_{lc}_{hf}")
                    for c in range(2 * hf, 2 * hf + 2):
                        nc.tensor.matmul(
                            psh[:, (c % 2) * 256:(c % 2) * 256 + 256],
                            lhsT=tT[c][:, lc * 128:(lc + 1) * 128],
                            rhs=d2p2[:, b * 1024 + c * 256:
                                     b * 1024 + (c + 1) * 256],
                            start=True, stop=True,
                        )
                    nc.scalar.activation(
                        Et[:, hf * 512:(hf + 1) * 512], psh[:], AF.Exp)
            else:
                ps = ps_main.tile([128, 2 * 512], FP32, tag="psm",
                                  name=f"ps_Et_{b}_{lc}")
                for c in range(NC4):
                    nc.tensor.matmul(
                        ps[:, c * 256:(c + 1) * 256],
                        lhsT=tT[c][:, lc * 128:(lc + 1) * 128],
                        rhs=d2p2[:, b * 1024 + c * 256:b * 1024 + (c + 1) * 256],
                        start=True, stop=True,
                    )
                nc.scalar.activation(Et[:], ps[:], AF.Exp)
            # colsum: pairwise add on GpSimd (otherwise idle), reduce on DVE
            t1 = junk.tile([128, 1024], BF16, tag="junk", name=f"t1_{b}_{lc}")
            v = Et[:].rearrange("p (s n) -> p s n", n=128)
            nc.gpsimd.tensor_tensor(
                t1[:, 0:512].rearrange("p (s n) -> p s n", n=64),
                v[:, :, 0:64], v[:, :, 64:128], ALU.add)
            nc.vector.tensor_reduce(
                S_t[:, lc * 8:(lc + 1) * 8],
                t1[:, 0:512].rearrange("p (s n) -> p s n", n=64),
                axis=mybir.AxisListType.X, op=ALU.add,
            )
            Et_tiles.append(Et)

        # g = tmw / (colsum + corr), column-doubled into the w_t matmul's
        # stationary operand; emitted in groups so the final chain is short
        G_GROUPS = {3: (0, 4), 5: (4, 6), 7: (6, 8)}

        def g_group(lc):
            lo, hi = G_GROUPS[lc]
            sl = slice(lo * 8, hi * 8)
            nc.vector.tensor_scalar(
                out=S_t[:, sl], in0=S_t[:, sl],
                scalar1=stcorrT[:, b:b + 1], scalar2=None, op0=ALU.add,
            )
            recipSt = small.tile([128, 32], FP32, tag="rSt",
                                 name=f"rSt_{b}_{lo}")
            g_half = small.tile([128, 32], FP32, tag="gh",
                                name=f"gh_{b}_{lo}")
            nc.vector.reciprocal(recipSt[:, 0:(hi - lo) * 8], S_t[:, sl])
            nc.vector.tensor_tensor(
                g_half[:, 0:(hi - lo) * 8], recipSt[:, 0:(hi - lo) * 8],
                tmw64_t[b][:, sl], ALU.mult)
            for lcc in range(lo, hi):
                gdual = gd_pool.tile([128, 512], BF16, tag="gd",
                                     name=f"gd_{b}_{lcc}")
                nc.vector.tensor_copy(
                    gdual[:].rearrange("p (h z) -> p h z", z=64),
                    g_half[:, (lcc - lo) * 8:(lcc - lo) * 8 + 8, None]
                    .to_broadcast((128, 8, 64)),
                )
                # w_t accumulation: valid quadrants (j<64, s=0) and
                # (j>=64, s=1). start=True only on the first MM touching
                # each bank (it clears has_written bank-wide); all other
                # MMs are ordered after it explicitly.
                for c in range(NC4):
                    st = (lcc == 0 and c % 2 == 0)
                    sp = (lcc == 7 and c % 2 == 1)
                    mm = nc.tensor.matmul(
                        ps_wt[:, c * 256:(c + 1) * 256],
                        lhsT=gdual[:, 2 * c * 64:2 * c * 64 + 128],
                        rhs=Et_tiles[lcc][:, c * 256:(c + 1) * 256],
                        start=st, stop=sp, skip_group_check=True,
                    )
                    bank = c // 2
                    if st:
                        mm_bank_start[bank] = mm
                    else:
                        add_dep_helper(
                            mm.ins, mm_bank_start[bank].ins, sync=False,
                            reason="ps_wt accum after bank-clearing MM")

        if nxt is not None:
            # ---- E phase (ACT-limited) with t-proj(b+1) slots 0,1 ----
            for h in range(H):
                e_head(h)
                if h in (0, 4):
                    t = tproj_group(nxt, tgtT_nxt, {0: 0, 4: 1}[h])
                    if t is not None:
                        tT_nxt.append(t)
            u_chain()
            # ---- Et phase with w_d, t-proj slots 2..7, g groups ----
            for lc in range(8):
                et_lc(lc)
                # last w_d group at lc=5, not 6: keeps its (slow) DVE fold
                # off the psum-rotation path at the batch boundary
                wd_at = {0: 0, 2: 1, 4: 2, 5: 3}.get(lc)
                if wd_at is not None:
                    wd_group(wd_at)
                slot = {1: 2, 2: 3, 3: 4, 5: 5, 6: 6, 7: 7}.get(lc)
                if slot is not None:
                    t = tproj_group(nxt, tgtT_nxt, slot)
                    if t is not None:
                        tT_nxt.append(t)
                if lc in G_GROUPS:
                    g_group(lc)
        else:
            # ---- last batch: no projection filler, so interleave the two
            # independent exp chains to keep ACT saturated ----
            for i in range(H):
                e_head(i)
                et_lc(i, tail=True)
                if i in G_GROUPS:
                    g_group(i)
            u_chain()
            for c in range(NC4):
                wd_group(c)

        # ---- ctx_t folds (two half-partition STTs per chunk) ----
        for c in range(NC4):
            scratch = junk.tile([128, 1024], BF16, tag="junk")
            nc.vector.scalar_tensor_tensor(
                out=scratch[0:64, 0:128],
                in0=ps_wt[0:64, c * 256:c * 256 + 128], scalar=1.0,
                in1=d2T_all[0:64, c * 512 + b * 128:c * 512 + (b + 1) * 128],
                op0=ALU.mult, op1=ALU.mult,
                accum_out=ctxv[0:64, 4 + c:5 + c],
            )
            nc.vector.scalar_tensor_tensor(
                out=scratch[64:128, 0:128],
                in0=ps_wt[64:128, c * 256 + 128:(c + 1) * 256], scalar=1.0,
                in1=d2T_all[64:128, c * 512 + b * 128:c * 512 + (b + 1) * 128],
                op0=ALU.mult, op1=ALU.mult,
                accum_out=ctxv[64:128, 4 + c:5 + c],
            )

        # ---------- transpose ctx [128, 8] -> [8, 128] and ship ----------
        ps_c = ps_acc.tile([128, 1024], FP32, tag="psa", name=f"ps_c_{b}")
        nc.tensor.transpose(ps_c[0:8, 0:128], ctxv[:], ident_f[:])
        nc.scalar.copy(ctxT_all[b * 32:b * 32 + 8, :], ps_c[0:8, 0:128])
        nc.sync.dma_start(
            out=io["out"][b].rearrange("(j p) -> j p", j=8),
            in_=ctxT_all[b * 32:b * 32 + 8, :],
        )

        if nxt is not None:
            tT_cur = tT_nxt
    es.close()


def _build():
    nc = bacc.Bacc("TRN2", target_bir_lowering=False, debug=False,
                   num_devices=NCORES)
    io = {}

    def inp(name, shape, dt):
        io[name] = nc.dram_tensor(name, shape, dt, kind="ExternalInput").ap()

    inp("tgtT", [BC, KT, L], BF16)
    inp("drug_pack", [128, NKC_D * BC * N], BF16)
    inp("wt", [KT, HID], BF16)
    inp("wd2_pack", [128, NKC_D * 2 * HID], BF16)
    inp("cpack", [128, 3 * BC + BC * 64], FP32)
    inp("rpack", [1, HID + 2 * HID + BC * L + BC * N], BF16)
    io["out"] = nc.dram_tensor("out", [BC, 2 * HID], FP32,
                               kind="ExternalOutput").ap()
    with tile.TileContext(nc) as tc:
        _body(tc, io)
    nc.compile()
    return nc


_NC_CACHE = None
_LAST_RESULTS = None


def _prep_host(drug_nodes, drug_mask, target_seq, target_mask,
               Wd, bd, Wt, bt, Wb):
    f32 = np.float32
    WdWb = np.einsum("khd,hde->khe", Wd.reshape(KD, H, HD), Wb).reshape(KD, HID)
    bdWb = np.einsum("hd,hde->he", bd.reshape(H, HD), Wb).reshape(HID)
    wd2 = np.ascontiguousarray(
        np.concatenate([Wd, WdWb], axis=1)).astype(BF16NP)
    wt_bf = np.ascontiguousarray(Wt).astype(BF16NP)
    b2 = np.concatenate([bd, bdWb]).astype(f32)
    dlen = np.maximum(drug_mask.sum(-1), 1).astype(f32)
    tlen = np.maximum(target_mask.sum(-1), 1).astype(f32)
    dmw = (drug_mask.astype(f32) / dlen[:, None]).astype(f32)
    tmw = (target_mask.astype(f32) / tlen[:, None]).astype(f32)
    sdcorr = (1e-30 - (L - tlen)).astype(f32)      # subtract invalid-l count
    stcorr = (1e-30 - (N - dlen)).astype(f32)      # subtract invalid-n count
    return wd2, wt_bf, b2, dmw, tmw, sdcorr, stcorr


def kernel(drug_nodes, drug_mask, target_seq, target_mask,
           Wd, bd, Wt, bt, Wb):
    f32 = np.float32
    drug_nodes = np.asarray(drug_nodes, f32)
    drug_mask = np.asarray(drug_mask)
    target_seq = np.asarray(target_seq, f32)
    target_mask = np.asarray(target_mask)
    Wd, bd = np.asarray(Wd, f32), np.asarray(bd, f32)
    Wt, bt = np.asarray(Wt, f32), np.asarray(bt, f32)
    Wb = np.asarray(Wb, f32)

    (wd2, wt_bf, b2, dmw, tmw, sdcorr, stcorr) = _prep_host(
        drug_nodes, drug_mask, target_seq, target_mask, Wd, bd, Wt, bt, Wb)
    # wd2_pack[p, kc*1024 + h] = wd2[kc*128+p, h]
    wd2_pack = np.ascontiguousarray(
        wd2.reshape(NKC_D, 128, 2 * HID).transpose(1, 0, 2)
        .reshape(128, NKC_D * 2 * HID))

    tgt_bf = (target_seq * target_mask[:, :, None]).astype(BF16NP)
    tgtT_h = np.ascontiguousarray(tgt_bf.transpose(0, 2, 1))
    drug_bf = (drug_nodes * drug_mask[:, :, None]).astype(BF16NP)
    drugT_h = np.ascontiguousarray(drug_bf.transpose(0, 2, 1))
    bt_bf = bt.astype(BF16NP)
    b2_bf = b2.astype(BF16NP)
    tmask_bf = target_mask.astype(f32).astype(BF16NP)
    dmask_bf = drug_mask.astype(f32).astype(BF16NP)
    tmw64 = np.repeat(
        tmw.reshape(B, 8, 128).transpose(0, 2, 1), 8, axis=2)

    in_maps = []
    for i in range(NCORES):
        s = slice(i * BC, (i + 1) * BC)
        cpack = np.empty((128, 3 * BC + BC * 64), f32)
        cpack[:, 0:BC] = dmw[s].T
        cpack[:, BC:2 * BC] = sdcorr[s][None, :]
        cpack[:, 2 * BC:3 * BC] = stcorr[s][None, :]
        cpack[:, 3 * BC:] = tmw64[s].transpose(1, 0, 2).reshape(128, BC * 64)
        rpack = np.concatenate(
            [bt_bf, b2_bf, tmask_bf[s].ravel(), dmask_bf[s].ravel()])[None, :]
        # drug_pack[p, kc*512 + b*128 + n] = drugT[b, kc*128+p, n]
        drug_pack = np.ascontiguousarray(
            drugT_h[s].reshape(BC, NKC_D, 128, N)
            .transpose(2, 1, 0, 3).reshape(128, NKC_D * BC * N))
        in_maps.append(dict(
            tgtT=np.ascontiguousarray(tgtT_h[s]),
            drug_pack=drug_pack,
            wt=wt_bf, wd2_pack=wd2_pack,
            cpack=np.ascontiguousarray(cpack),
            rpack=np.ascontiguousarray(rpack),
        ))

    nc = _get_nc()
    res = run_bass_kernel_spmd(nc, in_maps, list(range(NCORES)))
    global _LAST_RESULTS
    _LAST_RESULTS = res
    out = np.concatenate([res.results[i]["out"] for i in range(NCORES)],
                         axis=0)
    return np.ascontiguousarray(out.astype(np.float32))


def _get_nc():
    global _NC_CACHE
    if _NC_CACHE is None:
        _NC_CACHE = _build()
    return _NC_CACHE
